# revision 10
# baseline (speedup 1.0000x reference)
"""Trainium2 Bass kernel for nn_ConsistencyConstraint (loss_fn).

Reference computation (B=4096, D=C*H*W=4096, NCLASS=10):
    ngrad_i = (g_i - min_i) / (max_i - min_i)          per-row min-max norm
    vn_i    = ngrad_i / max(||ngrad_i||, eps)
    sim     = vn @ vn.T
    xloss   = sum_{i<j, pred_i==pred_j} (1 - sim_ij) / B
    celoss  = mean cross-entropy(outputs, y)
    loss    = celoss + xloss

Restructuring (mathematically identical up to fp32r PE rounding):

1. Cosine similarity is invariant to the per-row positive scale 1/(max-min),
   so vn_i = z_i / ||z_i|| with z_i = g_i - m_i, m_i = row min (the eps clamp
   is inactive: min-max normalized rows always have norm >= 1).
2. For same-class pairs: sum_{i<j in c} vn_i.vn_j = (||S_c||^2 - n_c) / 2
   where S_c = sum_{i in c} vn_i and sum_c n_c = B, so
       xloss = (N_pairs - (xsum - B) / 2) / B,   xsum = sum_c ||S_c||^2.
3. The shift by m_i is pulled OUT of the device matmul:
       S_c = sum_i eq_ic rs_i (g_i - m_i 1) = M_c - t_c 1,
       M   = Wa^T G   (Wa[i,c] = eq_ic rs_i),
       t_c = sum_i eq_ic m_i rs_i.
   The PE streams the raw grad as float32r (1 col/cycle for 512-wide moving
   blocks, ~bf16x2 precision) so NO cast / z materialization pass is needed;
   the host applies the rank-1 correction t_c from a tiny [128,KCH] output
   holding (-m)*rs.

Per-core device dataflow (512 rows, 4 chunks of 128 partitions):
  - DVE:  negated row min per DMA piece (tensor_reduce negate=True), merged
          with maxes -> the ACT bias -m with no extra negation ops. The last
          chunk arrives as half+quarter+eighth+eighth so only a ~0.6us
          reduce remains after the final byte. Also wa = onehot * rs
          (tensor_scalar_mul to float32r) and the (-m)*rs output.
  - ACT:  ssq = sum((g - m)^2) in ONE pass (Square, bias=-m, free-dim
          accumulate); rs = Rsqrt(ssq) directly (raw InstActivation: bass
          gates Rsqrt for accuracy, measured 4e-5 rel err on HW, fine at
          this tolerance). Square/Rsqrt/Copy share one activation table.
  - PE:   4 two-bank PSUM tiles accumulate M = Wa^T @ G over the 4 chunks
          (f32r moving at 1 col/cycle).
  - Drain: four [10,1024] PSUM->SBUF copies alternating DVE/ACT, overlapped
    with the tail matmuls; P goes out in two pieces, each dispatched as soon
    as its copies land.

Host: pred/one-hot/bincount/celoss from `outputs`/`y` (tiny [4096,10]);
xsum from gathered M and t; final scalar combine. No device collectives.
"""

import numpy as np

import concourse.bass as bass
import concourse.mybir as mybir
import concourse.tile as tile
from concourse import bacc
from concourse.bass_utils import run_bass_kernel_spmd

N_CORES = 8
B = 4096
D = 4096  # C*H*W = 1*64*64
NCLASS = 10
ROWS_PER_CORE = B // N_CORES  # 512
P = 128  # SBUF partitions
KCH = ROWS_PER_CORE // P  # 4 row-chunks per core
NFREE = 512  # PSUM bank width (fp32)
NCH = D // NFREE  # 8 column-chunks
DH = D // 2

F32 = mybir.dt.float32
F32R = mybir.dt.float32r
FP16 = mybir.dt.float16

MIN = mybir.AluOpType.min
MAX = mybir.AluOpType.max
MULT = mybir.AluOpType.mult

LAST_RESULTS = None
_nc_cache = None


def _act_raw(engine, out, in_, func, accum_out=None):
    """Emit InstActivation directly (bass refuses Rsqrt; its accuracy
    (~4e-5 rel) is far inside this kernel's tolerance)."""
    bias = engine.bass.const_aps.scalar_like(0.0, in_)
    ins = [engine.lower_ap(in_), engine.lower_ap(bias)]
    for arg in (1.0, 0.0):  # scale, alpha
        ins.append(mybir.ImmediateValue(dtype=mybir.dt.float32, value=arg))
    outs = [engine.lower_ap(out)]
    if accum_out is not None:
        outs.append(engine.lower_ap(accum_out))
    return engine.add_instruction(
        mybir.InstActivation(
            name=engine.bass.get_next_instruction_name(),
            func=func,
            ins=ins,
            outs=outs,
        )
    )


def _build_bass():
    """One SPMD program, identical on all 8 cores; only the data differs."""
    nc = bacc.Bacc()

    # g is fp32 bits; declared float32r end-to-end so the PE can stream it
    # at 1 col/cycle (the BIR verifier requires the producer to be f32r).
    g_in = nc.dram_tensor("g", [ROWS_PER_CORE, D], F32R, kind="ExternalInput")
    # host-computed argmax one-hot, fp16, [p, k*10+c] = onehot[k*128+p, c]
    eq_in = nc.dram_tensor("eq", [P, KCH * NCLASS], FP16, kind="ExternalInput")

    p_out = nc.dram_tensor("P", [NCLASS, D], F32, kind="ExternalOutput")
    # nmrs[p, k] = (-m) * rs for row k*128+p (host negates for t_c)
    nmrs_out = nc.dram_tensor("nmrs", [P, KCH], F32, kind="ExternalOutput")

    # per-chunk DMA pieces: the last chunk lands as half+quarter+2 eighths
    # so the tail's final min reduce is only 512 columns
    Q, E = D // 4, D // 8
    pieces = {k: (slice(0, DH), slice(DH, D)) for k in range(KCH)}
    pieces[KCH - 1] = (
        slice(0, DH),
        slice(DH, DH + Q),
        slice(DH + Q, DH + Q + E),
        slice(DH + Q + E, D),
    )

    with tile.TileContext(nc) as tc:
        with (
            tc.tile_pool(name="gpool", bufs=4) as gpool,
            tc.tile_pool(name="jpool", bufs=2) as jpool,
            tc.tile_pool(name="small", bufs=4) as small,
            tc.tile_pool(name="singles", bufs=1) as singles,
            tc.tile_pool(name="outp", bufs=1) as outp,
            tc.tile_pool(name="psum", bufs=1, space="PSUM") as psum,
        ):
            gts = []
            for k in range(KCH):
                gt = gpool.tile([P, D], F32R, tag="gt", name=f"gt{k}")
                for cols in pieces[k]:
                    nc.sync.dma_start(
                        out=gt[:, cols], in_=g_in[k * P : (k + 1) * P, cols]
                    )
                gts.append(gt)
                if k == 0:
                    # tiny one-hot input, dispatched after the first chunk
                    eq_sb = singles.tile([P, KCH, NCLASS], FP16)
                    nc.sync.dma_start(
                        out=eq_sb,
                        in_=eq_in.rearrange("p (k c) -> p k c", c=NCLASS),
                    )

            # persistent per-row scalars, one column per chunk
            negm_all = singles.tile([P, KCH], F32)  # -m (ACT bias)
            ssq_all = singles.tile([P, KCH], F32)
            rs_all = singles.tile([P, KCH], F32)  # 1/||z||  (ACT Rsqrt out)
            nmh_all = singles.tile([P, KCH, 3], F32)  # per-piece -min scratch
            nmrs_sb = outp.tile([P, KCH], F32)
            p_sb = outp.tile([NCLASS, D], F32)

            was = [
                small.tile([P, NCLASS], F32R, tag=f"wa{k}", name=f"wa{k}")
                for k in range(KCH)
            ]

            # 4 PSUM tiles, each spanning two physical banks
            acc = [
                psum.tile([NCLASS, 2 * NFREE], F32, tag=f"acc{j}", name=f"acc{j}")
                for j in range(NCH // 2)
            ]

            for k in range(KCH):
                gtf = gts[k].bitcast(F32)
                negm = negm_all[:, k : k + 1]
                ssq = ssq_all[:, k : k + 1]

                # negated min per DMA piece; merged via max -> negm = -m
                cols = pieces[k]
                for h, cs in enumerate(cols):
                    dst = negm if h == 0 else nmh_all[:, k, h - 1 : h]
                    nc.vector.tensor_reduce(
                        dst, gtf[:, cs], axis=mybir.AxisListType.X,
                        op=MIN, negate=True,
                    )
                    if h == 1 or h == 3:
                        # pairwise merge as soon as a pair exists
                        a = negm if h == 1 else nmh_all[:, k, 1:2]
                        nc.vector.tensor_tensor(
                            a, a, nmh_all[:, k, h - 1 : h], op=MAX
                        )
                if len(cols) == 3:
                    nc.vector.tensor_tensor(
                        negm, negm, nmh_all[:, k, 1:2], op=MAX
                    )
                elif len(cols) == 4:
                    nc.vector.tensor_tensor(
                        negm, negm, nmh_all[:, k, 1:2], op=MAX
                    )

                # ssq = sum((g - m)^2) in one ACT pass (junk fp16 main out),
                # then rs = 1/sqrt(ssq) directly
                junk = jpool.tile([P, D], FP16, tag="junk")
                nc.scalar.activation(
                    junk,
                    gtf,
                    mybir.ActivationFunctionType.Square,
                    bias=negm,
                    accum_out=ssq,
                )
                _act_raw(
                    nc.scalar,
                    rs_all[:, k : k + 1],
                    ssq,
                    mybir.ActivationFunctionType.Rsqrt,
                )

                # deferred in the DVE stream: wa_{k-1} after chunk k's mins,
                # so DVE never stalls waiting on ACT mid-stream
                if k > 0:
                    nc.vector.tensor_scalar_mul(
                        was[k - 1], eq_sb[:, k - 1, :], rs_all[:, k - 1 : k]
                    )

            nc.vector.tensor_scalar_mul(
                was[KCH - 1], eq_sb[:, KCH - 1, :], rs_all[:, KCH - 1 : KCH]
            )

            for k in range(KCH):
                for n in range(NCH):
                    nc.tensor.matmul(
                        acc[n // 2][:, (n % 2) * NFREE : (n % 2 + 1) * NFREE],
                        was[k],
                        gts[k][:, n * NFREE : (n + 1) * NFREE],
                        start=(k == 0),
                        stop=(k == KCH - 1),
                    )

            # (-m)*rs for the host-side rank-1 correction t_c
            nc.vector.tensor_tensor(nmrs_sb, negm_all, rs_all, op=MULT)
            nc.sync.dma_start(out=nmrs_out[:, :], in_=nmrs_sb)

            # drain PSUM -> SBUF as four 2-bank copies alternating DVE/ACT,
            # overlapped with the tail matmuls; P leaves in two pieces, each
            # dispatched as soon as its copies land
            for j in range(NCH // 2):
                dst = p_sb[:, j * 2 * NFREE : (j + 1) * 2 * NFREE]
                if j % 2 == 0:
                    nc.vector.tensor_copy(dst, acc[j])
                else:
                    nc.scalar.copy(dst, acc[j])
                if j == 1:
                    nc.sync.dma_start(
                        out=p_out[:, : 2 * 1024], in_=p_sb[:, : 2 * 1024]
                    )
            nc.sync.dma_start(out=p_out[:, 2 * 1024 :], in_=p_sb[:, 2 * 1024 :])

    nc.compile()
    return nc


def kernel(**inputs) -> np.ndarray:
    global LAST_RESULTS, _nc_cache

    outputs = np.asarray(inputs["outputs"], dtype=np.float32)
    grad = np.asarray(inputs["grad"], dtype=np.float32).reshape(B, D)
    y = np.asarray(inputs["y"]).astype(np.int64)

    if _nc_cache is None:
        _nc_cache = _build_bass()
    nc = _nc_cache

    # host: predicted class + one-hot (tiny [B, NCLASS])
    pred = np.argmax(outputs, axis=1)
    onehot = np.zeros((B, NCLASS), dtype=np.float16)
    onehot[np.arange(B), pred] = 1.0

    in_maps = []
    for c in range(N_CORES):
        sl = slice(c * ROWS_PER_CORE, (c + 1) * ROWS_PER_CORE)
        # [p, k*10+c] = onehot[k*128+p, c]
        eq_core = (
            onehot[sl]
            .reshape(KCH, P, NCLASS)
            .transpose(1, 0, 2)
            .reshape(P, KCH * NCLASS)
        )
        in_maps.append(
            {
                "g": np.ascontiguousarray(grad[sl]),
                "eq": np.ascontiguousarray(eq_core),
            }
        )

    res = run_bass_kernel_spmd(nc, in_maps, core_ids=list(range(N_CORES)))
    LAST_RESULTS = res
    results = res.results

    # ---- host gather / unshard ----
    m_full = np.zeros((NCLASS, D), dtype=np.float64)
    t_full = np.zeros(NCLASS, dtype=np.float64)
    for c, r in enumerate(results):
        m_full += r["P"].astype(np.float64)
        # nmrs[p, k] = (-m)*rs for row c*512 + k*128 + p
        mrs = -r["nmrs"].astype(np.float64).T.reshape(-1)  # [k*128+p]
        pc = pred[c * ROWS_PER_CORE : (c + 1) * ROWS_PER_CORE]
        t_full += np.bincount(pc, weights=mrs, minlength=NCLASS)

    s_full = m_full - t_full[:, None]
    xsum = float((s_full * s_full).sum())

    counts = np.bincount(pred, minlength=NCLASS).astype(np.float64)
    n_pairs = float((counts * (counts - 1) / 2).sum())
    xloss = (n_pairs - (xsum - B) / 2.0) / B

    # host cross-entropy in float64
    o = outputs.astype(np.float64)
    mo = o.max(axis=1)
    se = np.exp(o - mo[:, None]).sum(axis=1)
    celoss = float((np.log(se) + mo - o[np.arange(B), y]).mean())

    return np.float32(celoss + xloss)


# revision 11
# speedup vs baseline: 1.0941x; 1.0941x over previous
"""Trainium2 Bass kernel for nn_ConsistencyConstraint (loss_fn).

Reference computation (B=4096, D=C*H*W=4096, NCLASS=10):
    ngrad_i = (g_i - min_i) / (max_i - min_i)          per-row min-max norm
    vn_i    = ngrad_i / max(||ngrad_i||, eps)
    sim     = vn @ vn.T
    xloss   = sum_{i<j, pred_i==pred_j} (1 - sim_ij) / B
    celoss  = mean cross-entropy(outputs, y)
    loss    = celoss + xloss

Restructuring (mathematically identical up to fp32r PE rounding):

1. Cosine similarity is invariant to the per-row positive scale 1/(max-min),
   so vn_i = z_i / ||z_i|| with z_i = g_i - m_i, m_i = row min (the eps clamp
   is inactive: min-max normalized rows always have norm >= 1).
2. For same-class pairs: sum_{i<j in c} vn_i.vn_j = (||S_c||^2 - n_c) / 2
   where S_c = sum_{i in c} vn_i and sum_c n_c = B, so
       xloss = (N_pairs - (xsum - B) / 2) / B,   xsum = sum_c ||S_c||^2.
3. The shift by m_i is pulled OUT of the device matmul:
       S_c = sum_i eq_ic rs_i (g_i - m_i 1) = M_c - t_c 1,
       M   = Wa^T G   (Wa[i,c] = eq_ic rs_i),
       t_c = sum_i eq_ic m_i rs_i.
   The PE streams the raw grad as float32r (1 col/cycle for 512-wide moving
   blocks, ~bf16x2 precision) so NO cast / z materialization pass is needed;
   the host applies the rank-1 correction t_c from a tiny [128,KCH] output
   holding (-m)*rs.

Per-core device dataflow (512 rows, 4 chunks of 128 partitions):
  - DVE:  negated row min per DMA piece (tensor_reduce negate=True), merged
          with maxes -> the ACT bias -m with no extra negation ops. The last
          chunk arrives as half+quarter+eighth+eighth so only a ~0.6us
          reduce remains after the final byte. Also wa = onehot * rs
          (tensor_scalar_mul to float32r) and the (-m)*rs output.
  - ACT:  ssq = sum((g - m)^2) in ONE pass (Square, bias=-m, free-dim
          accumulate); rs = Rsqrt(ssq) directly (raw InstActivation: bass
          gates Rsqrt for accuracy, measured 4e-5 rel err on HW, fine at
          this tolerance). Square/Rsqrt/Copy share one activation table.
  - PE:   4 two-bank PSUM tiles accumulate M = Wa^T @ G over the 4 chunks
          (f32r moving at 1 col/cycle).
  - Drain: four [10,1024] PSUM->SBUF copies alternating DVE/ACT, overlapped
    with the tail matmuls; P goes out in two pieces, each dispatched as soon
    as its copies land.

Host: pred/one-hot/bincount/celoss from `outputs`/`y` (tiny [4096,10]);
xsum from gathered M and t; final scalar combine. No device collectives.
"""

import numpy as np

import concourse.bass as bass
import concourse.mybir as mybir
import concourse.tile as tile
from concourse import bacc
from concourse.bass_utils import run_bass_kernel_spmd

N_CORES = 8
B = 4096
D = 4096  # C*H*W = 1*64*64
NCLASS = 10
ROWS_PER_CORE = B // N_CORES  # 512
P = 128  # SBUF partitions
KCH = ROWS_PER_CORE // P  # 4 row-chunks per core
NFREE = 512  # PSUM bank width (fp32)
NCH = D // NFREE  # 8 column-chunks
DH = D // 2

F32 = mybir.dt.float32
F32R = mybir.dt.float32r
FP16 = mybir.dt.float16

MIN = mybir.AluOpType.min
MAX = mybir.AluOpType.max
MULT = mybir.AluOpType.mult

LAST_RESULTS = None
_nc_cache = None


def _act_raw(engine, out, in_, func, accum_out=None):
    """Emit InstActivation directly (bass refuses Rsqrt; its accuracy
    (~4e-5 rel) is far inside this kernel's tolerance)."""
    bias = engine.bass.const_aps.scalar_like(0.0, in_)
    ins = [engine.lower_ap(in_), engine.lower_ap(bias)]
    for arg in (1.0, 0.0):  # scale, alpha
        ins.append(mybir.ImmediateValue(dtype=mybir.dt.float32, value=arg))
    outs = [engine.lower_ap(out)]
    if accum_out is not None:
        outs.append(engine.lower_ap(accum_out))
    return engine.add_instruction(
        mybir.InstActivation(
            name=engine.bass.get_next_instruction_name(),
            func=func,
            ins=ins,
            outs=outs,
        )
    )


def _build_bass():
    """One SPMD program, identical on all 8 cores; only the data differs."""
    nc = bacc.Bacc()

    # g is staged by the host as fp16 (same PE precision class as the
    # original fp16-z baseline) -> the HBM stream is half the bytes.
    g_in = nc.dram_tensor("g", [ROWS_PER_CORE, D], FP16, kind="ExternalInput")
    # host-computed argmax one-hot, fp16, [p, k*10+c] = onehot[k*128+p, c]
    eq_in = nc.dram_tensor("eq", [P, KCH * NCLASS], FP16, kind="ExternalInput")

    p_out = nc.dram_tensor("P", [NCLASS, D], F32, kind="ExternalOutput")
    # nmrs[p, k] = (-m) * rs for row k*128+p (host negates for t_c)
    nmrs_out = nc.dram_tensor("nmrs", [P, KCH], F32, kind="ExternalOutput")

    # per-chunk DMA pieces: the last chunk lands as half+quarter+2 eighths
    # so the tail's final min reduce is only 512 columns
    Q, E = D // 4, D // 8
    pieces = {k: (slice(0, DH), slice(DH, D)) for k in range(KCH)}
    pieces[KCH - 1] = (
        slice(0, DH),
        slice(DH, DH + Q),
        slice(DH + Q, DH + Q + E),
        slice(DH + Q + E, D),
    )

    with tile.TileContext(nc) as tc:
        with (
            tc.tile_pool(name="gpool", bufs=4) as gpool,
            tc.tile_pool(name="jpool", bufs=2) as jpool,
            tc.tile_pool(name="small", bufs=4) as small,
            tc.tile_pool(name="singles", bufs=1) as singles,
            tc.tile_pool(name="outp", bufs=1) as outp,
            tc.tile_pool(name="psum", bufs=1, space="PSUM") as psum,
        ):
            gts = []
            for k in range(KCH):
                gt = gpool.tile([P, D], FP16, tag="gt", name=f"gt{k}")
                for cols in pieces[k]:
                    nc.sync.dma_start(
                        out=gt[:, cols], in_=g_in[k * P : (k + 1) * P, cols]
                    )
                gts.append(gt)
                if k == 0:
                    # tiny one-hot input, dispatched after the first chunk
                    eq_sb = singles.tile([P, KCH, NCLASS], FP16)
                    nc.sync.dma_start(
                        out=eq_sb,
                        in_=eq_in.rearrange("p (k c) -> p k c", c=NCLASS),
                    )

            # persistent per-row scalars, one column per chunk
            negm_all = singles.tile([P, KCH], F32)  # -m (ACT bias)
            ssq_all = singles.tile([P, KCH], F32)
            rs_all = singles.tile([P, KCH], F32)  # 1/||z||  (ACT Rsqrt out)
            nmh_all = singles.tile([P, KCH, 3], F32)  # per-piece -min scratch
            nmrs_sb = outp.tile([P, KCH], F32)
            p_sb = outp.tile([NCLASS, D], F32)

            was = [
                small.tile([P, NCLASS], FP16, tag=f"wa{k}", name=f"wa{k}")
                for k in range(KCH)
            ]

            # 4 PSUM tiles, each spanning two physical banks
            acc = [
                psum.tile([NCLASS, 2 * NFREE], F32, tag=f"acc{j}", name=f"acc{j}")
                for j in range(NCH // 2)
            ]

            for k in range(KCH):
                gtf = gts[k]
                negm = negm_all[:, k : k + 1]
                ssq = ssq_all[:, k : k + 1]

                # negated min per DMA piece; merged via max -> negm = -m
                cols = pieces[k]
                for h, cs in enumerate(cols):
                    dst = negm if h == 0 else nmh_all[:, k, h - 1 : h]
                    nc.vector.tensor_reduce(
                        dst, gtf[:, cs], axis=mybir.AxisListType.X,
                        op=MIN, negate=True,
                    )
                    if h == 1 or h == 3:
                        # pairwise merge as soon as a pair exists
                        a = negm if h == 1 else nmh_all[:, k, 1:2]
                        nc.vector.tensor_tensor(
                            a, a, nmh_all[:, k, h - 1 : h], op=MAX
                        )
                if len(cols) == 3:
                    nc.vector.tensor_tensor(
                        negm, negm, nmh_all[:, k, 1:2], op=MAX
                    )
                elif len(cols) == 4:
                    nc.vector.tensor_tensor(
                        negm, negm, nmh_all[:, k, 1:2], op=MAX
                    )

                # ssq = sum((g - m)^2) in one ACT pass (junk fp16 main out),
                # then rs = 1/sqrt(ssq) directly
                junk = jpool.tile([P, D], FP16, tag="junk")
                nc.scalar.activation(
                    junk,
                    gtf,
                    mybir.ActivationFunctionType.Square,
                    bias=negm,
                    accum_out=ssq,
                )
                _act_raw(
                    nc.scalar,
                    rs_all[:, k : k + 1],
                    ssq,
                    mybir.ActivationFunctionType.Rsqrt,
                )

                # deferred in the DVE stream: wa_{k-1} after chunk k's mins,
                # so DVE never stalls waiting on ACT mid-stream
                if k > 0:
                    nc.vector.tensor_scalar_mul(
                        was[k - 1], eq_sb[:, k - 1, :], rs_all[:, k - 1 : k]
                    )

            nc.vector.tensor_scalar_mul(
                was[KCH - 1], eq_sb[:, KCH - 1, :], rs_all[:, KCH - 1 : KCH]
            )

            for k in range(KCH):
                for n in range(NCH):
                    nc.tensor.matmul(
                        acc[n // 2][:, (n % 2) * NFREE : (n % 2 + 1) * NFREE],
                        was[k],
                        gts[k][:, n * NFREE : (n + 1) * NFREE],
                        start=(k == 0),
                        stop=(k == KCH - 1),
                    )

            # (-m)*rs for the host-side rank-1 correction t_c
            nc.vector.tensor_tensor(nmrs_sb, negm_all, rs_all, op=MULT)
            nc.sync.dma_start(out=nmrs_out[:, :], in_=nmrs_sb)

            # drain PSUM -> SBUF as four 2-bank copies alternating DVE/ACT,
            # overlapped with the tail matmuls; P leaves in two pieces, each
            # dispatched as soon as its copies land
            for j in range(NCH // 2):
                dst = p_sb[:, j * 2 * NFREE : (j + 1) * 2 * NFREE]
                if j % 2 == 0:
                    nc.vector.tensor_copy(dst, acc[j])
                else:
                    nc.scalar.copy(dst, acc[j])
                if j == 1:
                    nc.sync.dma_start(
                        out=p_out[:, : 2 * 1024], in_=p_sb[:, : 2 * 1024]
                    )
            nc.sync.dma_start(out=p_out[:, 2 * 1024 :], in_=p_sb[:, 2 * 1024 :])

    nc.compile()
    return nc


def kernel(**inputs) -> np.ndarray:
    global LAST_RESULTS, _nc_cache

    outputs = np.asarray(inputs["outputs"], dtype=np.float32)
    grad = np.asarray(inputs["grad"], dtype=np.float32).reshape(B, D)
    grad16 = grad.astype(np.float16)
    y = np.asarray(inputs["y"]).astype(np.int64)

    if _nc_cache is None:
        _nc_cache = _build_bass()
    nc = _nc_cache

    # host: predicted class + one-hot (tiny [B, NCLASS])
    pred = np.argmax(outputs, axis=1)
    onehot = np.zeros((B, NCLASS), dtype=np.float16)
    onehot[np.arange(B), pred] = 1.0

    in_maps = []
    for c in range(N_CORES):
        sl = slice(c * ROWS_PER_CORE, (c + 1) * ROWS_PER_CORE)
        # [p, k*10+c] = onehot[k*128+p, c]
        eq_core = (
            onehot[sl]
            .reshape(KCH, P, NCLASS)
            .transpose(1, 0, 2)
            .reshape(P, KCH * NCLASS)
        )
        in_maps.append(
            {
                "g": np.ascontiguousarray(grad16[sl]),
                "eq": np.ascontiguousarray(eq_core),
            }
        )

    res = run_bass_kernel_spmd(nc, in_maps, core_ids=list(range(N_CORES)))
    LAST_RESULTS = res
    results = res.results

    # ---- host gather / unshard ----
    m_full = np.zeros((NCLASS, D), dtype=np.float64)
    t_full = np.zeros(NCLASS, dtype=np.float64)
    for c, r in enumerate(results):
        m_full += r["P"].astype(np.float64)
        # nmrs[p, k] = (-m)*rs for row c*512 + k*128 + p
        mrs = -r["nmrs"].astype(np.float64).T.reshape(-1)  # [k*128+p]
        pc = pred[c * ROWS_PER_CORE : (c + 1) * ROWS_PER_CORE]
        t_full += np.bincount(pc, weights=mrs, minlength=NCLASS)

    s_full = m_full - t_full[:, None]
    xsum = float((s_full * s_full).sum())

    counts = np.bincount(pred, minlength=NCLASS).astype(np.float64)
    n_pairs = float((counts * (counts - 1) / 2).sum())
    xloss = (n_pairs - (xsum - B) / 2.0) / B

    # host cross-entropy in float64
    o = outputs.astype(np.float64)
    mo = o.max(axis=1)
    se = np.exp(o - mo[:, None]).sum(axis=1)
    celoss = float((np.log(se) + mo - o[np.arange(B), y]).mean())

    return np.float32(celoss + xloss)


# revision 12
# speedup vs baseline: 1.1614x; 1.0615x over previous
"""Trainium2 Bass kernel for nn_ConsistencyConstraint (loss_fn).

Reference computation (B=4096, D=C*H*W=4096, NCLASS=10):
    ngrad_i = (g_i - min_i) / (max_i - min_i)          per-row min-max norm
    vn_i    = ngrad_i / max(||ngrad_i||, eps)
    sim     = vn @ vn.T
    xloss   = sum_{i<j, pred_i==pred_j} (1 - sim_ij) / B
    celoss  = mean cross-entropy(outputs, y)
    loss    = celoss + xloss

Restructuring (mathematically identical up to fp32r PE rounding):

1. Cosine similarity is invariant to the per-row positive scale 1/(max-min),
   so vn_i = z_i / ||z_i|| with z_i = g_i - m_i, m_i = row min (the eps clamp
   is inactive: min-max normalized rows always have norm >= 1).
2. For same-class pairs: sum_{i<j in c} vn_i.vn_j = (||S_c||^2 - n_c) / 2
   where S_c = sum_{i in c} vn_i and sum_c n_c = B, so
       xloss = (N_pairs - (xsum - B) / 2) / B,   xsum = sum_c ||S_c||^2.
3. The shift by m_i is pulled OUT of the device matmul:
       S_c = sum_i eq_ic rs_i (g_i - m_i 1) = M_c - t_c 1,
       M   = Wa^T G   (Wa[i,c] = eq_ic rs_i),
       t_c = sum_i eq_ic m_i rs_i.
   The PE streams the raw grad as float32r (1 col/cycle for 512-wide moving
   blocks, ~bf16x2 precision) so NO cast / z materialization pass is needed;
   the host applies the rank-1 correction t_c from a tiny [128,KCH] output
   holding (-m)*rs.

Per-core device dataflow (512 rows, 4 chunks of 128 partitions):
  - DVE:  negated row min per DMA piece (tensor_reduce negate=True), merged
          with maxes -> the ACT bias -m with no extra negation ops. The last
          chunk arrives as half+quarter+eighth+eighth so only a ~0.6us
          reduce remains after the final byte. Also wa = onehot * rs
          (tensor_scalar_mul to float32r) and the (-m)*rs output.
  - ACT:  ssq = sum((g - m)^2) in ONE pass (Square, bias=-m, free-dim
          accumulate); rs = Rsqrt(ssq) directly (raw InstActivation: bass
          gates Rsqrt for accuracy, measured 4e-5 rel err on HW, fine at
          this tolerance). Square/Rsqrt/Copy share one activation table.
  - PE:   4 two-bank PSUM tiles accumulate M = Wa^T @ G over the 4 chunks
          (f32r moving at 1 col/cycle).
  - Drain: four [10,1024] PSUM->SBUF copies alternating DVE/ACT, overlapped
    with the tail matmuls; P goes out in two pieces, each dispatched as soon
    as its copies land.

Host: pred/one-hot/bincount/celoss from `outputs`/`y` (tiny [4096,10]);
xsum from gathered M and t; final scalar combine. No device collectives.
"""

import numpy as np

import concourse.bass as bass
import concourse.mybir as mybir
import concourse.tile as tile
from concourse import bacc
from concourse.bass_utils import run_bass_kernel_spmd

N_CORES = 8
B = 4096
D = 4096  # C*H*W = 1*64*64
NCLASS = 10
ROWS_PER_CORE = B // N_CORES  # 512
P = 128  # SBUF partitions
KCH = ROWS_PER_CORE // P  # 4 row-chunks per core
NFREE = 512  # PSUM bank width (fp32)
NCH = D // NFREE  # 8 column-chunks
DH = D // 2

F32 = mybir.dt.float32
F32R = mybir.dt.float32r
FP16 = mybir.dt.float16

MIN = mybir.AluOpType.min
MAX = mybir.AluOpType.max
MULT = mybir.AluOpType.mult

LAST_RESULTS = None
_nc_cache = None


def _act_raw(engine, out, in_, func, accum_out=None):
    """Emit InstActivation directly (bass refuses Rsqrt; its accuracy
    (~4e-5 rel) is far inside this kernel's tolerance)."""
    bias = engine.bass.const_aps.scalar_like(0.0, in_)
    ins = [engine.lower_ap(in_), engine.lower_ap(bias)]
    for arg in (1.0, 0.0):  # scale, alpha
        ins.append(mybir.ImmediateValue(dtype=mybir.dt.float32, value=arg))
    outs = [engine.lower_ap(out)]
    if accum_out is not None:
        outs.append(engine.lower_ap(accum_out))
    return engine.add_instruction(
        mybir.InstActivation(
            name=engine.bass.get_next_instruction_name(),
            func=func,
            ins=ins,
            outs=outs,
        )
    )


def _build_bass():
    """One SPMD program, identical on all 8 cores; only the data differs."""
    nc = bacc.Bacc()

    # g is staged by the host as fp16 (same PE precision class as the
    # original fp16-z baseline) -> the HBM stream is half the bytes.
    g_in = nc.dram_tensor("g", [ROWS_PER_CORE, D], FP16, kind="ExternalInput")
    # host-computed argmax one-hot, fp16, [p, k*10+c] = onehot[k*128+p, c]
    eq_in = nc.dram_tensor("eq", [P, KCH * NCLASS], FP16, kind="ExternalInput")

    p_out = nc.dram_tensor("P", [NCLASS, D], F32, kind="ExternalOutput")
    # nmrs[p, k] = (-m) * rs for row k*128+p (host negates for t_c)
    nmrs_out = nc.dram_tensor("nmrs", [P, KCH], F32, kind="ExternalOutput")

    # per-chunk DMA pieces: the last chunk lands as half+quarter+2 eighths
    # so the tail's final min reduce is only 512 columns
    Q, E = D // 4, D // 8
    pieces = {k: (slice(0, DH), slice(DH, D)) for k in range(KCH)}
    pieces[KCH - 1] = (
        slice(0, DH),
        slice(DH, DH + Q),
        slice(DH + Q, DH + Q + E),
        slice(DH + Q + E, D),
    )

    with tile.TileContext(nc) as tc:
        with (
            tc.tile_pool(name="gpool", bufs=4) as gpool,
            tc.tile_pool(name="jpool", bufs=2) as jpool,
            tc.tile_pool(name="small", bufs=4) as small,
            tc.tile_pool(name="singles", bufs=1) as singles,
            tc.tile_pool(name="outp", bufs=1) as outp,
            tc.tile_pool(name="psum", bufs=1, space="PSUM") as psum,
        ):
            gts = []
            for k in range(KCH):
                gt = gpool.tile([P, D], FP16, tag="gt", name=f"gt{k}")
                for cols in pieces[k]:
                    nc.sync.dma_start(
                        out=gt[:, cols], in_=g_in[k * P : (k + 1) * P, cols]
                    )
                gts.append(gt)
                if k == 0:
                    # tiny one-hot input, dispatched after the first chunk
                    eq_sb = singles.tile([P, KCH, NCLASS], FP16)
                    nc.sync.dma_start(
                        out=eq_sb,
                        in_=eq_in.rearrange("p (k c) -> p k c", c=NCLASS),
                    )

            # persistent per-row scalars, one column per chunk
            negm_all = singles.tile([P, KCH], F32)  # -m (ACT bias)
            ssq_all = singles.tile([P, KCH], F32)
            rs_all = singles.tile([P, KCH], F32)  # 1/||z||  (ACT Rsqrt out)
            nmh_all = singles.tile([P, KCH, 3], F32)  # per-piece -min scratch
            nmrs_sb = outp.tile([P, KCH], F32)
            p_sb = outp.tile([NCLASS, D], F32)

            was = [
                small.tile([P, NCLASS], FP16, tag=f"wa{k}", name=f"wa{k}")
                for k in range(KCH)
            ]

            # 4 PSUM tiles, each spanning two physical banks
            acc = [
                psum.tile([NCLASS, 2 * NFREE], F32, tag=f"acc{j}", name=f"acc{j}")
                for j in range(NCH // 2)
            ]

            for k in range(KCH):
                gtf = gts[k]
                negm = negm_all[:, k : k + 1]
                ssq = ssq_all[:, k : k + 1]

                # negated min per DMA piece, as a tt-min tree (the fp16
                # elementwise min runs in the 2x DVE mode; the final short
                # reduce is 1x), merged via max -> negm = -m
                cols = pieces[k]
                for h, cs in enumerate(cols):
                    dst = negm if h == 0 else nmh_all[:, k, h - 1 : h]
                    width = cs.stop - cs.start
                    src = gtf[:, cs]
                    while width > 512:
                        width //= 2
                        stage = small.tile(
                            [P, width], FP16, tag=f"mt{h}_{width}",
                            name=f"mt{h}_{width}",
                        )
                        nc.vector.tensor_tensor(
                            stage, src[:, :width], src[:, width:], op=MIN
                        )
                        src = stage
                    nc.vector.tensor_reduce(
                        dst, src, axis=mybir.AxisListType.X,
                        op=MIN, negate=True,
                    )
                    if h == 1 or h == 3:
                        # pairwise merge as soon as a pair exists
                        a = negm if h == 1 else nmh_all[:, k, 1:2]
                        nc.vector.tensor_tensor(
                            a, a, nmh_all[:, k, h - 1 : h], op=MAX
                        )
                if len(cols) == 4:
                    nc.vector.tensor_tensor(
                        negm, negm, nmh_all[:, k, 1:2], op=MAX
                    )

                # ssq = sum((g - m)^2) in one ACT pass (junk fp16 main out),
                # then rs = 1/sqrt(ssq) directly
                junk = jpool.tile([P, D], FP16, tag="junk")
                nc.scalar.activation(
                    junk,
                    gtf,
                    mybir.ActivationFunctionType.Square,
                    bias=negm,
                    accum_out=ssq,
                )
                _act_raw(
                    nc.scalar,
                    rs_all[:, k : k + 1],
                    ssq,
                    mybir.ActivationFunctionType.Rsqrt,
                )

                # deferred in the DVE stream: wa_{k-1} after chunk k's mins,
                # so DVE never stalls waiting on ACT mid-stream
                if k > 0:
                    nc.vector.tensor_scalar_mul(
                        was[k - 1], eq_sb[:, k - 1, :], rs_all[:, k - 1 : k]
                    )

            nc.vector.tensor_scalar_mul(
                was[KCH - 1], eq_sb[:, KCH - 1, :], rs_all[:, KCH - 1 : KCH]
            )

            for k in range(KCH):
                for n in range(NCH):
                    nc.tensor.matmul(
                        acc[n // 2][:, (n % 2) * NFREE : (n % 2 + 1) * NFREE],
                        was[k],
                        gts[k][:, n * NFREE : (n + 1) * NFREE],
                        start=(k == 0),
                        stop=(k == KCH - 1),
                    )

            # (-m)*rs for the host-side rank-1 correction t_c
            nc.vector.tensor_tensor(nmrs_sb, negm_all, rs_all, op=MULT)
            nc.sync.dma_start(out=nmrs_out[:, :], in_=nmrs_sb)

            # drain PSUM -> SBUF as four 2-bank copies alternating DVE/ACT,
            # overlapped with the tail matmuls; P leaves in two pieces, each
            # dispatched as soon as its copies land
            for j in range(NCH // 2):
                dst = p_sb[:, j * 2 * NFREE : (j + 1) * 2 * NFREE]
                if j % 2 == 0:
                    nc.vector.tensor_copy(dst, acc[j])
                else:
                    nc.scalar.copy(dst, acc[j])
                if j == 1:
                    nc.sync.dma_start(
                        out=p_out[:, : 2 * 1024], in_=p_sb[:, : 2 * 1024]
                    )
            nc.sync.dma_start(out=p_out[:, 2 * 1024 :], in_=p_sb[:, 2 * 1024 :])

    nc.compile()
    return nc


def kernel(**inputs) -> np.ndarray:
    global LAST_RESULTS, _nc_cache

    outputs = np.asarray(inputs["outputs"], dtype=np.float32)
    grad = np.asarray(inputs["grad"], dtype=np.float32).reshape(B, D)
    grad16 = grad.astype(np.float16)
    y = np.asarray(inputs["y"]).astype(np.int64)

    if _nc_cache is None:
        _nc_cache = _build_bass()
    nc = _nc_cache

    # host: predicted class + one-hot (tiny [B, NCLASS])
    pred = np.argmax(outputs, axis=1)
    onehot = np.zeros((B, NCLASS), dtype=np.float16)
    onehot[np.arange(B), pred] = 1.0

    in_maps = []
    for c in range(N_CORES):
        sl = slice(c * ROWS_PER_CORE, (c + 1) * ROWS_PER_CORE)
        # [p, k*10+c] = onehot[k*128+p, c]
        eq_core = (
            onehot[sl]
            .reshape(KCH, P, NCLASS)
            .transpose(1, 0, 2)
            .reshape(P, KCH * NCLASS)
        )
        in_maps.append(
            {
                "g": np.ascontiguousarray(grad16[sl]),
                "eq": np.ascontiguousarray(eq_core),
            }
        )

    res = run_bass_kernel_spmd(nc, in_maps, core_ids=list(range(N_CORES)))
    LAST_RESULTS = res
    results = res.results

    # ---- host gather / unshard ----
    m_full = np.zeros((NCLASS, D), dtype=np.float64)
    t_full = np.zeros(NCLASS, dtype=np.float64)
    for c, r in enumerate(results):
        m_full += r["P"].astype(np.float64)
        # nmrs[p, k] = (-m)*rs for row c*512 + k*128 + p
        mrs = -r["nmrs"].astype(np.float64).T.reshape(-1)  # [k*128+p]
        pc = pred[c * ROWS_PER_CORE : (c + 1) * ROWS_PER_CORE]
        t_full += np.bincount(pc, weights=mrs, minlength=NCLASS)

    s_full = m_full - t_full[:, None]
    xsum = float((s_full * s_full).sum())

    counts = np.bincount(pred, minlength=NCLASS).astype(np.float64)
    n_pairs = float((counts * (counts - 1) / 2).sum())
    xloss = (n_pairs - (xsum - B) / 2.0) / B

    # host cross-entropy in float64
    o = outputs.astype(np.float64)
    mo = o.max(axis=1)
    se = np.exp(o - mo[:, None]).sum(axis=1)
    celoss = float((np.log(se) + mo - o[np.arange(B), y]).mean())

    return np.float32(celoss + xloss)


# revision 13
# speedup vs baseline: 1.2162x; 1.0472x over previous
"""Trainium2 Bass kernel for nn_ConsistencyConstraint (loss_fn).

Reference computation (B=4096, D=C*H*W=4096, NCLASS=10):
    ngrad_i = (g_i - min_i) / (max_i - min_i)          per-row min-max norm
    vn_i    = ngrad_i / max(||ngrad_i||, eps)
    sim     = vn @ vn.T
    xloss   = sum_{i<j, pred_i==pred_j} (1 - sim_ij) / B
    celoss  = mean cross-entropy(outputs, y)
    loss    = celoss + xloss

Restructuring (mathematically identical up to fp32r PE rounding):

1. Cosine similarity is invariant to the per-row positive scale 1/(max-min),
   so vn_i = z_i / ||z_i|| with z_i = g_i - m_i, m_i = row min (the eps clamp
   is inactive: min-max normalized rows always have norm >= 1).
2. For same-class pairs: sum_{i<j in c} vn_i.vn_j = (||S_c||^2 - n_c) / 2
   where S_c = sum_{i in c} vn_i and sum_c n_c = B, so
       xloss = (N_pairs - (xsum - B) / 2) / B,   xsum = sum_c ||S_c||^2.
3. The shift by m_i is pulled OUT of the device matmul:
       S_c = sum_i eq_ic rs_i (g_i - m_i 1) = M_c - t_c 1,
       M   = Wa^T G   (Wa[i,c] = eq_ic rs_i),
       t_c = sum_i eq_ic m_i rs_i.
   The PE streams the raw grad as float32r (1 col/cycle for 512-wide moving
   blocks, ~bf16x2 precision) so NO cast / z materialization pass is needed;
   the host applies the rank-1 correction t_c from a tiny [128,KCH] output
   holding (-m)*rs.

Per-core device dataflow (512 rows, 4 chunks of 128 partitions):
  - DVE:  negated row min per DMA piece (tensor_reduce negate=True), merged
          with maxes -> the ACT bias -m with no extra negation ops. The last
          chunk arrives as half+quarter+eighth+eighth so only a ~0.6us
          reduce remains after the final byte. Also wa = onehot * rs
          (tensor_scalar_mul to float32r) and the (-m)*rs output.
  - ACT:  ssq = sum((g - m)^2) in ONE pass (Square, bias=-m, free-dim
          accumulate); rs = Rsqrt(ssq) directly (raw InstActivation: bass
          gates Rsqrt for accuracy, measured 4e-5 rel err on HW, fine at
          this tolerance). Square/Rsqrt/Copy share one activation table.
  - PE:   4 two-bank PSUM tiles accumulate M = Wa^T @ G over the 4 chunks
          (f32r moving at 1 col/cycle).
  - Drain: four [10,1024] PSUM->SBUF copies alternating DVE/ACT, overlapped
    with the tail matmuls; P goes out in two pieces, each dispatched as soon
    as its copies land.

Host: pred/one-hot/bincount/celoss from `outputs`/`y` (tiny [4096,10]);
xsum from gathered M and t; final scalar combine. No device collectives.
"""

import numpy as np

import concourse.bass as bass
import concourse.mybir as mybir
import concourse.tile as tile
from concourse import bacc
from concourse.bass_utils import run_bass_kernel_spmd

N_CORES = 8
B = 4096
D = 4096  # C*H*W = 1*64*64
NCLASS = 10
ROWS_PER_CORE = B // N_CORES  # 512
P = 128  # SBUF partitions
KCH = ROWS_PER_CORE // P  # 4 row-chunks per core
NFREE = 512  # PSUM bank width (fp32)
NCH = D // NFREE  # 8 column-chunks
DH = D // 2

F32 = mybir.dt.float32
F32R = mybir.dt.float32r
FP16 = mybir.dt.float16

MIN = mybir.AluOpType.min
MAX = mybir.AluOpType.max
MULT = mybir.AluOpType.mult

LAST_RESULTS = None
_nc_cache = None


def _act_raw(engine, out, in_, func, accum_out=None):
    """Emit InstActivation directly (bass refuses Rsqrt; its accuracy
    (~4e-5 rel) is far inside this kernel's tolerance)."""
    bias = engine.bass.const_aps.scalar_like(0.0, in_)
    ins = [engine.lower_ap(in_), engine.lower_ap(bias)]
    for arg in (1.0, 0.0):  # scale, alpha
        ins.append(mybir.ImmediateValue(dtype=mybir.dt.float32, value=arg))
    outs = [engine.lower_ap(out)]
    if accum_out is not None:
        outs.append(engine.lower_ap(accum_out))
    return engine.add_instruction(
        mybir.InstActivation(
            name=engine.bass.get_next_instruction_name(),
            func=func,
            ins=ins,
            outs=outs,
        )
    )


def _build_bass():
    """One SPMD program, identical on all 8 cores; only the data differs."""
    nc = bacc.Bacc()

    # g is staged by the host as fp16 (same PE precision class as the
    # original fp16-z baseline) -> the HBM stream is half the bytes.
    g_in = nc.dram_tensor("g", [ROWS_PER_CORE, D], FP16, kind="ExternalInput")
    # host-computed argmax one-hot, fp16, [p, k*10+c] = onehot[k*128+p, c]
    eq_in = nc.dram_tensor("eq", [P, KCH * NCLASS], FP16, kind="ExternalInput")

    p_out = nc.dram_tensor("P", [NCLASS, D], F32, kind="ExternalOutput")
    # nmrs[p, k] = (-m) * rs for row k*128+p (host negates for t_c)
    nmrs_out = nc.dram_tensor("nmrs", [P, KCH], F32, kind="ExternalOutput")

    # per-chunk DMA pieces: the last chunk lands as half+quarter+2 eighths
    # so the tail's final min reduce is only 512 columns
    Q, E = D // 4, D // 8
    pieces = {k: (slice(0, DH), slice(DH, D)) for k in range(KCH)}
    pieces[KCH - 1] = (
        slice(0, DH),
        slice(DH, DH + Q),
        slice(DH + Q, DH + Q + E),
        slice(DH + Q + E, D),
    )

    with tile.TileContext(nc) as tc:
        with (
            tc.tile_pool(name="gpool", bufs=4) as gpool,
            tc.tile_pool(name="jpool", bufs=2) as jpool,
            tc.tile_pool(name="small", bufs=4) as small,
            tc.tile_pool(name="singles", bufs=1) as singles,
            tc.tile_pool(name="outp", bufs=1) as outp,
            tc.tile_pool(name="psum", bufs=1, space="PSUM") as psum,
        ):
            # pin the ACT table to reciprocal_sqrt_and_small (holds Square,
            # Rsqrt AND Copy) before any data arrives: the lazy loader then
            # never swaps tables mid-pipeline
            warm = singles.tile([P, 1], F32)
            nc.gpsimd.memset(warm, 1.0)
            warm2 = singles.tile([P, 1], F32)
            _act_raw(nc.scalar, warm2, warm, mybir.ActivationFunctionType.Rsqrt)

            gts = []
            for k in range(KCH):
                gt = gpool.tile([P, D], FP16, tag="gt", name=f"gt{k}")
                for cols in pieces[k]:
                    nc.sync.dma_start(
                        out=gt[:, cols], in_=g_in[k * P : (k + 1) * P, cols]
                    )
                gts.append(gt)
                if k == 1:
                    # tiny one-hot input, dispatched after two chunks
                    eq_sb = singles.tile([P, KCH, NCLASS], FP16)
                    nc.sync.dma_start(
                        out=eq_sb,
                        in_=eq_in.rearrange("p (k c) -> p k c", c=NCLASS),
                    )

            # persistent per-row scalars, one column per chunk
            negm_all = singles.tile([P, KCH], F32)  # -m (ACT bias)
            ssq_all = singles.tile([P, KCH], F32)
            rs_all = singles.tile([P, KCH], F32)  # 1/||z||  (ACT Rsqrt out)
            nmh_all = singles.tile([P, KCH, 3], F32)  # per-piece -min scratch
            nmrs_sb = outp.tile([P, KCH], F32)
            p_sb = outp.tile([NCLASS, D], F32)

            was = [
                small.tile([P, NCLASS], FP16, tag=f"wa{k}", name=f"wa{k}")
                for k in range(KCH)
            ]

            # 4 PSUM tiles, each spanning two physical banks
            acc = [
                psum.tile([NCLASS, 2 * NFREE], F32, tag=f"acc{j}", name=f"acc{j}")
                for j in range(NCH // 2)
            ]

            for k in range(KCH):
                gtf = gts[k]
                negm = negm_all[:, k : k + 1]
                ssq = ssq_all[:, k : k + 1]

                # negated min per DMA piece, as a tt-min tree (the fp16
                # elementwise min runs in the 2x DVE mode; the final short
                # reduce is 1x), merged via max -> negm = -m
                cols = pieces[k]
                for h, cs in enumerate(cols):
                    dst = negm if h == 0 else nmh_all[:, k, h - 1 : h]
                    width = cs.stop - cs.start
                    src = gtf[:, cs]
                    while width > 512:
                        width //= 2
                        stage = small.tile(
                            [P, width], FP16, tag=f"mt{h}_{width}",
                            name=f"mt{h}_{width}",
                        )
                        nc.vector.tensor_tensor(
                            stage, src[:, :width], src[:, width:], op=MIN
                        )
                        src = stage
                    nc.vector.tensor_reduce(
                        dst, src, axis=mybir.AxisListType.X,
                        op=MIN, negate=True,
                    )
                    if h == 1 or h == 3:
                        # pairwise merge as soon as a pair exists
                        a = negm if h == 1 else nmh_all[:, k, 1:2]
                        nc.vector.tensor_tensor(
                            a, a, nmh_all[:, k, h - 1 : h], op=MAX
                        )
                if len(cols) == 4:
                    nc.vector.tensor_tensor(
                        negm, negm, nmh_all[:, k, 1:2], op=MAX
                    )

                # ssq = sum((g - m)^2) in one ACT pass (junk fp16 main out),
                # then rs = 1/sqrt(ssq) directly
                junk = jpool.tile([P, D], FP16, tag="junk")
                nc.scalar.activation(
                    junk,
                    gtf,
                    mybir.ActivationFunctionType.Square,
                    bias=negm,
                    accum_out=ssq,
                )
                _act_raw(
                    nc.scalar,
                    rs_all[:, k : k + 1],
                    ssq,
                    mybir.ActivationFunctionType.Rsqrt,
                )

                # deferred in the DVE stream: wa_{k-1} after chunk k's mins,
                # so DVE never stalls waiting on ACT mid-stream
                if k > 0:
                    nc.vector.tensor_scalar_mul(
                        was[k - 1], eq_sb[:, k - 1, :], rs_all[:, k - 1 : k]
                    )

            nc.vector.tensor_scalar_mul(
                was[KCH - 1], eq_sb[:, KCH - 1, :], rs_all[:, KCH - 1 : KCH]
            )

            for k in range(KCH):
                for n in range(NCH):
                    nc.tensor.matmul(
                        acc[n // 2][:, (n % 2) * NFREE : (n % 2 + 1) * NFREE],
                        was[k],
                        gts[k][:, n * NFREE : (n + 1) * NFREE],
                        start=(k == 0),
                        stop=(k == KCH - 1),
                    )

            # (-m)*rs for the host-side rank-1 correction t_c
            nc.vector.tensor_tensor(nmrs_sb, negm_all, rs_all, op=MULT)
            nc.sync.dma_start(out=nmrs_out[:, :], in_=nmrs_sb)

            # drain PSUM -> SBUF as four 2-bank copies alternating DVE/ACT,
            # overlapped with the tail matmuls; P leaves in two pieces, each
            # dispatched as soon as its copies land
            for j in range(NCH // 2):
                dst = p_sb[:, j * 2 * NFREE : (j + 1) * 2 * NFREE]
                if j % 2 == 0:
                    nc.vector.tensor_copy(dst, acc[j])
                else:
                    nc.scalar.copy(dst, acc[j])
                if j == 1:
                    nc.sync.dma_start(
                        out=p_out[:, : 2 * 1024], in_=p_sb[:, : 2 * 1024]
                    )
            nc.sync.dma_start(out=p_out[:, 2 * 1024 :], in_=p_sb[:, 2 * 1024 :])

    nc.compile()
    return nc


def kernel(**inputs) -> np.ndarray:
    global LAST_RESULTS, _nc_cache

    outputs = np.asarray(inputs["outputs"], dtype=np.float32)
    grad = np.asarray(inputs["grad"], dtype=np.float32).reshape(B, D)
    grad16 = grad.astype(np.float16)
    y = np.asarray(inputs["y"]).astype(np.int64)

    if _nc_cache is None:
        _nc_cache = _build_bass()
    nc = _nc_cache

    # host: predicted class + one-hot (tiny [B, NCLASS])
    pred = np.argmax(outputs, axis=1)
    onehot = np.zeros((B, NCLASS), dtype=np.float16)
    onehot[np.arange(B), pred] = 1.0

    in_maps = []
    for c in range(N_CORES):
        sl = slice(c * ROWS_PER_CORE, (c + 1) * ROWS_PER_CORE)
        # [p, k*10+c] = onehot[k*128+p, c]
        eq_core = (
            onehot[sl]
            .reshape(KCH, P, NCLASS)
            .transpose(1, 0, 2)
            .reshape(P, KCH * NCLASS)
        )
        in_maps.append(
            {
                "g": np.ascontiguousarray(grad16[sl]),
                "eq": np.ascontiguousarray(eq_core),
            }
        )

    res = run_bass_kernel_spmd(nc, in_maps, core_ids=list(range(N_CORES)))
    LAST_RESULTS = res
    results = res.results

    # ---- host gather / unshard ----
    m_full = np.zeros((NCLASS, D), dtype=np.float64)
    t_full = np.zeros(NCLASS, dtype=np.float64)
    for c, r in enumerate(results):
        m_full += r["P"].astype(np.float64)
        # nmrs[p, k] = (-m)*rs for row c*512 + k*128 + p
        mrs = -r["nmrs"].astype(np.float64).T.reshape(-1)  # [k*128+p]
        pc = pred[c * ROWS_PER_CORE : (c + 1) * ROWS_PER_CORE]
        t_full += np.bincount(pc, weights=mrs, minlength=NCLASS)

    s_full = m_full - t_full[:, None]
    xsum = float((s_full * s_full).sum())

    counts = np.bincount(pred, minlength=NCLASS).astype(np.float64)
    n_pairs = float((counts * (counts - 1) / 2).sum())
    xloss = (n_pairs - (xsum - B) / 2.0) / B

    # host cross-entropy in float64
    o = outputs.astype(np.float64)
    mo = o.max(axis=1)
    se = np.exp(o - mo[:, None]).sum(axis=1)
    celoss = float((np.log(se) + mo - o[np.arange(B), y]).mean())

    return np.float32(celoss + xloss)
